# revision 39
# baseline (speedup 1.0000x reference)
"""CrossCoderDecoder forward on 8 trn2 NeuronCores.

x[b,l,d] = sum_f f[b,f] * weight[l,f,d] + bias[l,d]
B=32, L=2, F=65536, D=768, fp32.

Sharding: the F (dict) axis is split 8 ways (8192 features per core).
Each core computes its partial [L, B, D] sums; the host sums the 8
partials and adds the bias (the "all-reduce" of the sharding hint,
done host-side since the output is tiny).

Precision/perf scheme: the kernel is HBM-bound on streaming the weight
(L*FS*D elements/core, each used once), so bytes/element is the whole
game. Both f and weight are cast to SINGLE bf16 (2 B/elem vs fp32's
4): one streaming pass on the PE at 1 col/cyc, fp32 PSUM accumulate.
Total error ~2e-3 max-rel vs the 2e-2 gate.

Weight DMA layout: per chunk of R k-rows ONE dma_start moves a
contiguous [P, R/P, L*D] block (both l interleaved per k-row) into
SBUF. The (l,d) axis is flat: 1536 = 3x512, so each k-subtile is
exactly 3 full-width 512-col matmuls into 3 PSUM banks held open all
kernel. Chunk sizes taper up then down (256, 768, 6x1024, 512, 256,
128, 128 rows): 3.1 MB bulk transfers for DMA efficiency, small edge
chunks so the PE's first stationary load isn't gated on a bulk
transfer and the end-of-stream completion latency + final matmul
burst expose as little as possible.

ALL input DMAs (f, then every chunk, in exact consumption order) ride
the single SP/sync HWDGE queue: measured SDMA arbitration on this
machine effectively strict-prioritizes the SP queue (~390-412 GB/s
solo, even with the PE streaming concurrently) over the ACT queue
(~170-200 GB/s), so splitting chunks across both rings starves every
ACT-ring chunk and stalls the PE mid-stream (~91-105us vs ~80us).
The final PSUM drains split across the vector+scalar engines and the
single output DMA rides the empty ACT ring. A single SBUF tile pool
with per-tag buffer groups keeps the Tile semaphore/barrier overhead
(prologue+epilogue) down.

Host-side prep packs the weight into the exact SBUF images
(k = kofs(chunk) + p*(R/P) + o at image[p, o]) and permutes f into
fhl[p, j, b] with the matching k order, so the contraction stays
consistent.
"""

import contextlib

import numpy as np
import ml_dtypes

import concourse.bass as bass
import concourse.tile as tile
from concourse import bacc, mybir
from concourse import bass_utils

B, L, F, D = 32, 2, 65536, 768
NCORES = 8
FS = F // NCORES          # 8192 features per core
P = 128
CHUNKS = (256, 768, 1024, 1024, 1024, 1024, 1024, 1024, 512, 256, 128, 128)
W_BUFS = {8: 4, 6: 1, 4: 1, 2: 2, 1: 2}           # per-size-class bufs
ND = L * D                        # 1536 = 3 x 512: flat (l,d) axis
NSPL = 3                          # PSUM-bank splits of ND, 512 cols each

# Chunks streamed as SCALED fp8 e4m3 (1 B/elem) instead of bf16: the
# weight rows are pre-multiplied by FP8_SCALE (w ~ N(0, 1/F) is deep in
# e4m3's subnormal range unscaled) and the matching f columns divided
# by FP8_SCALE in bf16 (an exact exponent shift), so the product is
# unchanged. Each fp8 row carries ~3.6% element error vs bf16's 0.2%;
# with 16 of 64 k-subtiles (25% of the reduction) in fp8 the output
# error grows ~sqrt(fp8 fraction). Measured: 2 chunks (25%) -> 1.40e-2
# max-rel, 1 chunk (12.5%) keeps ~2x margin under the 2e-2 gate while
# still cutting 1.6 MB/core (~4us at 412 GB/s).
FP8_CHUNKS = (7,)
FP8_SCALE = 64.0

assert sum(CHUNKS) == FS
_KOS = [r // P for r in CHUNKS]                   # k-subtiles per chunk
_NJ = sum(_KOS)                                   # 64 subtiles
_KOS_BF = [ko for ci, ko in enumerate(_KOS) if ci not in FP8_CHUNKS]
_CLASSES = sorted(set(_KOS_BF), reverse=True)     # bf16 chunk sizes
_NQ = len(FP8_CHUNKS)
_KOQ = _KOS[FP8_CHUNKS[0]] if _NQ else 0
assert all(_KOS[ci] == _KOQ for ci in FP8_CHUNKS)

_F32 = mybir.dt.float32
_BF16 = mybir.dt.bfloat16
_FP8 = mybir.dt.float8e4
_BF16_NP = ml_dtypes.bfloat16
_FP8_NP = ml_dtypes.float8_e4m3

_cache = {}


def set_chunks(chunks: tuple, w_bufs: dict | None = None):
    """Adjust chunking (for tuning sweeps); drops the cached program."""
    global CHUNKS, _KOS, _NJ, _CLASSES
    CHUNKS = tuple(chunks)
    assert sum(CHUNKS) == FS
    _KOS = [r // P for r in CHUNKS]
    _NJ = sum(_KOS)
    _CLASSES = sorted(set(_KOS), reverse=True)
    if w_bufs is not None:
        W_BUFS.update(w_bufs)
    _cache.clear()


def _build():
    """Build + schedule the (per-core identical) Bass program once."""
    nc = bacc.Bacc("TRN2", target_bir_lowering=False, debug=False)

    fhl = nc.dram_tensor("fhl", [P, _NJ, B], _BF16, kind="ExternalInput").ap()
    wdram = {
        ko: nc.dram_tensor(
            f"w{ko}", [_KOS_BF.count(ko), P, ko, ND], _BF16, kind="ExternalInput"
        ).ap()
        for ko in _CLASSES
    }
    wq = (
        nc.dram_tensor("wq", [_NQ, P, _KOQ, ND], _FP8, kind="ExternalInput").ap()
        if _NQ
        else None
    )
    out = nc.dram_tensor("out", [B, ND], _F32, kind="ExternalOutput").ap()

    with tile.TileContext(nc) as tc:
        with (
            tc.tile_pool(name="sb", bufs=1) as sb,
            tc.tile_pool(name="psum", bufs=1, space="PSUM") as psum,
        ):
            # The whole input stream rides the SP (sync) HWDGE queue in
            # exact consumption order: on this machine the SDMA
            # arbitration effectively strict-prioritizes the SP queue
            # (~390 GB/s solo) over the ACT queue (~170-200 GB/s), so
            # splitting chunks across the two rings starves every
            # ACT-ring chunk and stalls the PE mid-stream. f first, then
            # chunks smallest-first (taper-up) so the PE starts early.
            f_sb = sb.tile([P, _NJ, B], _BF16, tag="f", bufs=1, name="f_sb")
            nc.sync.dma_start(f_sb[:], fhl[:])

            # The (l, d) axis is flattened to 1536 = 3 x 512 columns, so
            # each k-subtile takes exactly 3 full-width 512-col matmuls
            # into 3 PSUM banks (vs 4 matmuls over per-l 512+256 splits):
            # fewer PE dispatches, one less bank, one less drain copy.
            ps = [
                psum.tile([B, 512], _F32, name=f"ps_{i}") for i in range(NSPL)
            ]
            jofs = 0
            cls_idx = {ko: 0 for ko in _CLASSES}
            qi = 0
            for ci, r in enumerate(CHUNKS):
                ko = r // P
                if ci in FP8_CHUNKS:
                    wt = sb.tile(
                        [P, ko, ND], _FP8, tag="wq", bufs=_NQ, name=f"wq{ci}"
                    )
                    nc.sync.dma_start(wt[:], wq[qi])
                    qi += 1
                else:
                    wt = sb.tile(
                        [P, ko, ND], _BF16, tag=f"w{ko}", bufs=W_BUFS[ko],
                        name=f"wt{ci}",
                    )
                    nc.sync.dma_start(wt[:], wdram[ko][cls_idx[ko]])
                    cls_idx[ko] += 1
                for o in range(ko):
                    j = jofs + o
                    for i in range(NSPL):
                        nc.tensor.matmul(
                            ps[i][:],
                            f_sb[:, j, :],
                            wt[:, o, i * 512 : (i + 1) * 512],
                            start=(j == 0),
                            stop=(j == _NJ - 1),
                        )
                jofs += ko
            # Drain: copies split across the (otherwise idle) scalar
            # engine and vector, then one output DMA on the empty ACT
            # ring so it never queues behind the weight stream.
            out_sb = sb.tile([B, ND], _F32, tag="o", bufs=1, name="o_sb")
            nc.vector.tensor_copy(out=out_sb[:, 0:512], in_=ps[0][:])
            nc.scalar.copy(out=out_sb[:, 512:1024], in_=ps[1][:])
            nc.scalar.copy(out=out_sb[:, 1024:1536], in_=ps[2][:])
            nc.scalar.dma_start(out[:], out_sb[:])

    nc.compile()
    return nc


def _prep_core(f_core: np.ndarray, w_core: np.ndarray) -> dict:
    """Build the per-core input map.

    f_core [B, FS] fp32 -> fhl [P, NJ, B] bf16 with
    fhl[p, jofs+o, b] = f[b, kofs + p*ko + o] per chunk.
    w_core [L, FS, D] fp32 -> one [cnt, P, ko, L, D] bf16 image per
    chunk-size class, matching the kernel's DMA order.
    """
    fh = f_core.astype(_BF16_NP)
    wh = w_core.astype(_BF16_NP)          # [L, FS, D]
    whT = np.ascontiguousarray(wh.transpose(1, 0, 2))  # [FS, L, D]
    fhl = np.empty((P, _NJ, B), dtype=_BF16_NP)
    wimgs = {ko: [] for ko in _CLASSES}
    wqimgs = []
    kofs = 0
    jofs = 0
    for ci, r in enumerate(CHUNKS):
        ko = r // P
        # k = kofs + p*ko + o  (C-order reshape)
        if ci in FP8_CHUNKS:
            # w*SCALE into e4m3's normal range; f/SCALE is exact in bf16.
            fq = (fh[:, kofs : kofs + r].astype(np.float32) / FP8_SCALE)
            fhl[:, jofs : jofs + ko, :] = (
                fq.astype(_BF16_NP).T.reshape(P, ko, B)
            )
            wqimgs.append(
                (whT[kofs : kofs + r].astype(np.float32) * FP8_SCALE)
                .astype(_FP8_NP)
                .reshape(P, ko, ND)
            )
        else:
            fhl[:, jofs : jofs + ko, :] = (
                fh[:, kofs : kofs + r].T.reshape(P, ko, B)
            )
            wimgs[ko].append(whT[kofs : kofs + r].reshape(P, ko, ND))
        kofs += r
        jofs += ko
    in_map = {"fhl": np.ascontiguousarray(fhl)}
    for ko in _CLASSES:
        in_map[f"w{ko}"] = np.ascontiguousarray(np.stack(wimgs[ko]))
    if wqimgs:
        in_map["wq"] = np.ascontiguousarray(np.stack(wqimgs))
    return in_map


def kernel(f: np.ndarray, weight: np.ndarray, bias: np.ndarray) -> np.ndarray:
    f = np.asarray(f, dtype=np.float32)
    weight = np.asarray(weight, dtype=np.float32)
    bias = np.asarray(bias, dtype=np.float32)

    if "nc" not in _cache:
        _cache["nc"] = _build()
    nc = _cache["nc"]

    in_maps = []
    for c in range(NCORES):
        sl = slice(c * FS, (c + 1) * FS)
        in_maps.append(_prep_core(f[:, sl], weight[:, sl, :]))

    res = bass_utils.run_bass_kernel_spmd(nc, in_maps, core_ids=list(range(NCORES)))
    partial = np.stack([r["out"] for r in res.results])  # [NCORES, B, ND]
    total = partial.sum(axis=0).reshape(B, L, D)         # flat (l,d) -> [B, L, D]
    x = total + bias[None, :, :]
    return x.astype(np.float32)


# revision 40
# speedup vs baseline: 1.1227x; 1.1227x over previous
"""CrossCoderDecoder forward on 8 trn2 NeuronCores.

x[b,l,d] = sum_f f[b,f] * weight[l,f,d] + bias[l,d]
B=32, L=2, F=65536, D=768, fp32.

Sharding: the F (dict) axis is split 8 ways (8192 features per core).
Each core computes its partial [L, B, D] sums; the host sums the 8
partials and adds the bias (the "all-reduce" of the sharding hint,
done host-side since the output is tiny).

Precision/perf scheme: the kernel is HBM-bound on streaming the weight
(L*FS*D elements/core, each used once), so bytes/element is the whole
game. Both f and weight are cast to SINGLE bf16 (2 B/elem vs fp32's
4): one streaming pass on the PE at 1 col/cyc, fp32 PSUM accumulate.
Total error ~2e-3 max-rel vs the 2e-2 gate.

Weight DMA layout: per chunk of R k-rows ONE dma_start moves a
contiguous [P, R/P, L*D] block (both l interleaved per k-row) into
SBUF. The (l,d) axis is flat: 1536 = 3x512, so each k-subtile is
exactly 3 full-width 512-col matmuls into 3 PSUM banks held open all
kernel. Chunk sizes taper up then down (256, 768, 6x1024, 512, 256,
128, 128 rows): 3.1 MB bulk transfers for DMA efficiency, small edge
chunks so the PE's first stationary load isn't gated on a bulk
transfer and the end-of-stream completion latency + final matmul
burst expose as little as possible.

ALL input DMAs (f, then every chunk, in exact consumption order) ride
the single SP/sync HWDGE queue: measured SDMA arbitration on this
machine effectively strict-prioritizes the SP queue (~390-412 GB/s
solo, even with the PE streaming concurrently) over the ACT queue
(~170-200 GB/s), so splitting chunks across both rings starves every
ACT-ring chunk and stalls the PE mid-stream (~91-105us vs ~80us).
The final PSUM drains split across the vector+scalar engines and the
single output DMA rides the empty ACT ring. A single SBUF tile pool
with per-tag buffer groups keeps the Tile semaphore/barrier overhead
(prologue+epilogue) down.

Host-side prep packs the weight into the exact SBUF images
(k = kofs(chunk) + p*(R/P) + o at image[p, o]) and permutes f into
fhl[p, j, b] with the matching k order, so the contraction stays
consistent.
"""

import contextlib

import numpy as np
import ml_dtypes

import concourse.bass as bass
import concourse.tile as tile
from concourse import bacc, mybir
from concourse import bass_utils

B, L, F, D = 32, 2, 65536, 768
NCORES = 8
FS = F // NCORES          # 8192 features per core
P = 128
CHUNKS = (256, 768, 1024, 1024, 1024, 1024, 1024, 1024, 512, 256, 128, 128)
W_BUFS = {8: 4, 6: 1, 4: 1, 2: 2, 1: 2}           # per-size-class bufs
ND = L * D                        # 1536 = 3 x 512: flat (l,d) axis
NSPL = 3                          # PSUM-bank splits of ND, 512 cols each

# Chunks streamed as SCALED fp8 e4m3 (1 B/elem) instead of bf16: the
# weight rows are pre-multiplied by FP8_SCALE (w ~ N(0, 1/F) is deep in
# e4m3's subnormal range unscaled) and the matching f columns divided
# by FP8_SCALE in bf16 (an exact exponent shift), so the product is
# unchanged. Each fp8 row carries ~3.6% element error vs bf16's 0.2%;
# with 16 of 64 k-subtiles (25% of the reduction) in fp8 the output
# error grows ~sqrt(fp8 fraction). Measured: 2 chunks (25%) -> 1.40e-2
# max-rel (deterministic, 1.43x under the 2e-2 gate; 1 chunk -> 1.05e-2),
# cutting 3.1 MB/core (~7.5us at 412 GB/s).
FP8_CHUNKS = (6, 7)
FP8_SCALE = 64.0

assert sum(CHUNKS) == FS
_KOS = [r // P for r in CHUNKS]                   # k-subtiles per chunk
_NJ = sum(_KOS)                                   # 64 subtiles
_KOS_BF = [ko for ci, ko in enumerate(_KOS) if ci not in FP8_CHUNKS]
_CLASSES = sorted(set(_KOS_BF), reverse=True)     # bf16 chunk sizes
_NQ = len(FP8_CHUNKS)
_KOQ = _KOS[FP8_CHUNKS[0]] if _NQ else 0
assert all(_KOS[ci] == _KOQ for ci in FP8_CHUNKS)

_F32 = mybir.dt.float32
_BF16 = mybir.dt.bfloat16
_FP8 = mybir.dt.float8e4
_BF16_NP = ml_dtypes.bfloat16
_FP8_NP = ml_dtypes.float8_e4m3

_cache = {}


def set_chunks(chunks: tuple, w_bufs: dict | None = None):
    """Adjust chunking (for tuning sweeps); drops the cached program."""
    global CHUNKS, _KOS, _NJ, _CLASSES
    CHUNKS = tuple(chunks)
    assert sum(CHUNKS) == FS
    _KOS = [r // P for r in CHUNKS]
    _NJ = sum(_KOS)
    _CLASSES = sorted(set(_KOS), reverse=True)
    if w_bufs is not None:
        W_BUFS.update(w_bufs)
    _cache.clear()


def _build():
    """Build + schedule the (per-core identical) Bass program once."""
    nc = bacc.Bacc("TRN2", target_bir_lowering=False, debug=False)

    fhl = nc.dram_tensor("fhl", [P, _NJ, B], _BF16, kind="ExternalInput").ap()
    wdram = {
        ko: nc.dram_tensor(
            f"w{ko}", [_KOS_BF.count(ko), P, ko, ND], _BF16, kind="ExternalInput"
        ).ap()
        for ko in _CLASSES
    }
    wq = (
        nc.dram_tensor("wq", [_NQ, P, _KOQ, ND], _FP8, kind="ExternalInput").ap()
        if _NQ
        else None
    )
    out = nc.dram_tensor("out", [B, ND], _F32, kind="ExternalOutput").ap()

    with tile.TileContext(nc) as tc:
        with (
            tc.tile_pool(name="sb", bufs=1) as sb,
            tc.tile_pool(name="psum", bufs=1, space="PSUM") as psum,
        ):
            # The whole input stream rides the SP (sync) HWDGE queue in
            # exact consumption order: on this machine the SDMA
            # arbitration effectively strict-prioritizes the SP queue
            # (~390 GB/s solo) over the ACT queue (~170-200 GB/s), so
            # splitting chunks across the two rings starves every
            # ACT-ring chunk and stalls the PE mid-stream. f first, then
            # chunks smallest-first (taper-up) so the PE starts early.
            f_sb = sb.tile([P, _NJ, B], _BF16, tag="f", bufs=1, name="f_sb")
            nc.sync.dma_start(f_sb[:], fhl[:])

            # The (l, d) axis is flattened to 1536 = 3 x 512 columns, so
            # each k-subtile takes exactly 3 full-width 512-col matmuls
            # into 3 PSUM banks (vs 4 matmuls over per-l 512+256 splits):
            # fewer PE dispatches, one less bank, one less drain copy.
            ps = [
                psum.tile([B, 512], _F32, name=f"ps_{i}") for i in range(NSPL)
            ]
            jofs = 0
            cls_idx = {ko: 0 for ko in _CLASSES}
            qi = 0
            for ci, r in enumerate(CHUNKS):
                ko = r // P
                if ci in FP8_CHUNKS:
                    wt = sb.tile(
                        [P, ko, ND], _FP8, tag="wq", bufs=_NQ, name=f"wq{ci}"
                    )
                    nc.sync.dma_start(wt[:], wq[qi])
                    qi += 1
                else:
                    wt = sb.tile(
                        [P, ko, ND], _BF16, tag=f"w{ko}", bufs=W_BUFS[ko],
                        name=f"wt{ci}",
                    )
                    nc.sync.dma_start(wt[:], wdram[ko][cls_idx[ko]])
                    cls_idx[ko] += 1
                for o in range(ko):
                    j = jofs + o
                    for i in range(NSPL):
                        nc.tensor.matmul(
                            ps[i][:],
                            f_sb[:, j, :],
                            wt[:, o, i * 512 : (i + 1) * 512],
                            start=(j == 0),
                            stop=(j == _NJ - 1),
                        )
                jofs += ko
            # Drain: copies split across the (otherwise idle) scalar
            # engine and vector, then one output DMA on the empty ACT
            # ring so it never queues behind the weight stream.
            out_sb = sb.tile([B, ND], _F32, tag="o", bufs=1, name="o_sb")
            nc.vector.tensor_copy(out=out_sb[:, 0:512], in_=ps[0][:])
            nc.scalar.copy(out=out_sb[:, 512:1024], in_=ps[1][:])
            nc.scalar.copy(out=out_sb[:, 1024:1536], in_=ps[2][:])
            nc.scalar.dma_start(out[:], out_sb[:])

    nc.compile()
    return nc


def _prep_core(f_core: np.ndarray, w_core: np.ndarray) -> dict:
    """Build the per-core input map.

    f_core [B, FS] fp32 -> fhl [P, NJ, B] bf16 with
    fhl[p, jofs+o, b] = f[b, kofs + p*ko + o] per chunk.
    w_core [L, FS, D] fp32 -> one [cnt, P, ko, L, D] bf16 image per
    chunk-size class, matching the kernel's DMA order.
    """
    fh = f_core.astype(_BF16_NP)
    wh = w_core.astype(_BF16_NP)          # [L, FS, D]
    whT = np.ascontiguousarray(wh.transpose(1, 0, 2))  # [FS, L, D]
    fhl = np.empty((P, _NJ, B), dtype=_BF16_NP)
    wimgs = {ko: [] for ko in _CLASSES}
    wqimgs = []
    kofs = 0
    jofs = 0
    for ci, r in enumerate(CHUNKS):
        ko = r // P
        # k = kofs + p*ko + o  (C-order reshape)
        if ci in FP8_CHUNKS:
            # w*SCALE into e4m3's normal range; f/SCALE is exact in bf16.
            fq = (fh[:, kofs : kofs + r].astype(np.float32) / FP8_SCALE)
            fhl[:, jofs : jofs + ko, :] = (
                fq.astype(_BF16_NP).T.reshape(P, ko, B)
            )
            wqimgs.append(
                (whT[kofs : kofs + r].astype(np.float32) * FP8_SCALE)
                .astype(_FP8_NP)
                .reshape(P, ko, ND)
            )
        else:
            fhl[:, jofs : jofs + ko, :] = (
                fh[:, kofs : kofs + r].T.reshape(P, ko, B)
            )
            wimgs[ko].append(whT[kofs : kofs + r].reshape(P, ko, ND))
        kofs += r
        jofs += ko
    in_map = {"fhl": np.ascontiguousarray(fhl)}
    for ko in _CLASSES:
        in_map[f"w{ko}"] = np.ascontiguousarray(np.stack(wimgs[ko]))
    if wqimgs:
        in_map["wq"] = np.ascontiguousarray(np.stack(wqimgs))
    return in_map


def kernel(f: np.ndarray, weight: np.ndarray, bias: np.ndarray) -> np.ndarray:
    f = np.asarray(f, dtype=np.float32)
    weight = np.asarray(weight, dtype=np.float32)
    bias = np.asarray(bias, dtype=np.float32)

    if "nc" not in _cache:
        _cache["nc"] = _build()
    nc = _cache["nc"]

    in_maps = []
    for c in range(NCORES):
        sl = slice(c * FS, (c + 1) * FS)
        in_maps.append(_prep_core(f[:, sl], weight[:, sl, :]))

    res = bass_utils.run_bass_kernel_spmd(nc, in_maps, core_ids=list(range(NCORES)))
    partial = np.stack([r["out"] for r in res.results])  # [NCORES, B, ND]
    total = partial.sum(axis=0).reshape(B, L, D)         # flat (l,d) -> [B, L, D]
    x = total + bias[None, :, :]
    return x.astype(np.float32)
